# revision 23
# baseline (speedup 1.0000x reference)
"""MHC-lite block on 8x TRN2 NeuronCores — transfer-minimal hybrid split.

The link between host and the (axon-tunneled) devices moves ~20-45 MB/s,
so wall time is dominated by bytes on the wire, not device FLOPs.  The
inner FFN is ~98% of the FLOPs but only needs layer_input [8192,1024]
as input and returns ffn_out of the same shape.  Split accordingly:

  host   : rms-norm stats, the three tiny projections (x @ [4096,32]),
           gates/softmax, layer_input, and the final reconstruction
           y = H@x + h_post*delta — all cheap fp32 passes over x, which
           never leaves the host (better precision than device bf16 x).
  device : FFN only (1024 -> 4096 gelu -> 1024), data-parallel over
           tokens (1024 tokens/core).  FFN weights are baked into the
           NEFF as inline consts so they ship once at executable load.

Wire format is int8 with a per-token scale both ways (licensed by the
error budget: measured rel err ~1e-3 vs the 2e-2 gate): 8 MB up
(layer_input) + 8 MB down (ffn_out) + 32 KB scales each way.  The
dequant/requant runs on device (scalar/vector engines); the host does
one rowmax/rint pass.

The runner mirrors bass2jax.run_bass_via_pjrt but is built once and
cached: the jit object persists (no per-call retrace), and the donated
output buffers are created device-side by a cached jitted zeros fn
instead of shipping host zeros through the tunnel.
"""

import hashlib
import os
import time

import numpy as np
import ml_dtypes

import jax
import jax.numpy as jnp
from jax.experimental.shard_map import shard_map
from jax.sharding import Mesh, NamedSharding, PartitionSpec

import concourse.bacc as bacc
import concourse.mybir as mybir
import concourse.tile as tile
from concourse.bass2jax import (_bass_exec_p, install_neuronx_cc_hook,
                                partition_id_tensor)

N_CORES = 8
T_CORE = 1024          # tokens per core
NTOK = 8192            # total tokens
HID = 1024
NCH = 4096
DFF = 4096
EPS = 1.1920929e-07
QMAX = 126.0           # int8 quant range with headroom below 127

F32 = mybir.dt.float32
I8 = mybir.dt.int8
BF16 = mybir.dt.bfloat16
BF = ml_dtypes.bfloat16
MULT = mybir.AluOpType.mult

_STATE = {}
_DBG = bool(os.environ.get("KT_DEBUG"))


def _build_module(w1t, w2t, b1r, b2r):
    nc = bacc.Bacc("TRN2", target_bir_lowering=False, debug=False,
                   num_devices=N_CORES)
    li8_d = nc.dram_tensor("li8", [T_CORE, HID], I8, kind="ExternalInput").ap()
    lis_d = nc.dram_tensor("lis", [T_CORE, 1], F32, kind="ExternalInput").ap()
    out8_d = nc.dram_tensor("out8", [T_CORE, HID], I8,
                            kind="ExternalOutput").ap()
    outs_d = nc.dram_tensor("outs", [T_CORE, 1], F32,
                            kind="ExternalOutput").ap()
    w1c = nc.inline_tensor(w1t, name="w1c").ap()     # [1024, 4096] = w1.T
    w2c = nc.inline_tensor(w2t, name="w2c").ap()     # [4096, 1024] = w2.T
    b1c = nc.inline_tensor(b1r, name="b1c").ap()     # [128, 32]
    b2c = nc.inline_tensor(b2r, name="b2c").ap()     # [1, 1024]

    with tile.TileContext(nc, trace_sim=False) as tc:
        _emit(nc, tc, li8_d, lis_d, out8_d, outs_d, w1c, w2c, b1c, b2c)
    nc.compile()
    return nc


def _emit(nc, tc, li8_d, lis_d, out8_d, outs_d, w1c, w2c, b1c, b2c):
    pools = []

    def _pool(*a, **k):
        p = tc.alloc_tile_pool(*a, **k)
        pools.append(p)
        return p

    cp = _pool(name="const", bufs=1)
    w1_sb = cp.tile([128, 8 * DFF], BF16, tag="w1")
    w2_sb = cp.tile([128, 32 * HID], BF16, tag="w2")
    b1_sb = cp.tile([128, 32], F32, tag="b1")
    b2_sb = cp.tile([1, HID], BF16, tag="b2")
    ones_sb = cp.tile([1, 128], BF16, tag="ones")

    # w1_sb[p, k*DFF + d] = w1.T[k*128+p, d];  lhsT tile for (k, m) is
    # w1_sb[:, k*DFF + m*128 : k*DFF + (m+1)*128]
    for k in range(8):
        nc.sync.dma_start(w1_sb[:, k * DFF:(k + 1) * DFF],
                          w1c[k * 128:(k + 1) * 128, :])
    # w2_sb[p, m*HID + c] = w2.T[m*128+p, c]
    for m in range(32):
        nc.sync.dma_start(w2_sb[:, m * HID:(m + 1) * HID],
                          w2c[m * 128:(m + 1) * 128, :])
    nc.sync.dma_start(b1_sb[:, :], b1c[:, :])
    nc.sync.dma_start(b2_sb[:, :], b2c[:, :])
    nc.vector.memset(ones_sb[:, :], 1.0)

    l8p = _pool(name="li8", bufs=3)
    lqp = _pool(name="liq", bufs=3)
    lbp = _pool(name="libf", bufs=3)
    xtp = _pool(name="liT", bufs=2)
    hp = _pool(name="h", bufs=4)
    outp = _pool(name="out8", bufs=3)
    qp = _pool(name="q", bufs=3)
    psA = _pool(name="psA", bufs=4, space="PSUM")
    psB = _pool(name="psB", bufs=3, space="PSUM")

    for g in range(4):                    # groups of 256 tokens
        # liT[p, k*256 + t] = li[g*256 + t, k*128 + p]   (channel-major)
        liT = xtp.tile([128, 8 * 256], BF16, tag="liT")
        for ti in range(2):
            r0 = (2 * g + ti) * 128
            li8_t = l8p.tile([128, HID], I8, tag="li8")
            nc.sync.dma_start(li8_t[:, :], li8_d[r0:r0 + 128, :])
            lqs = lqp.tile([128, 1], F32, tag="liq")
            nc.sync.dma_start(lqs[:, :], lis_d[r0:r0 + 128, :])
            libf = lbp.tile([128, HID], BF16, tag="libf")
            nc.scalar.activation(libf[:, :], li8_t[:, :],
                                 mybir.ActivationFunctionType.Copy,
                                 scale=lqs[:, 0:1])
            for k in range(8):
                nc.sync.dma_start_transpose(
                    liT[:, k * 256 + ti * 128: k * 256 + ti * 128 + 128],
                    libf[:, k * 128:(k + 1) * 128])

        # out accumulators: [ti*2+hf] -> [128 tok, 512 hid]
        fps = [psA.tile([128, 512], F32, tag="psA", name=f"fps_{g}_{q}")
               for q in range(4)]
        for m in range(32):               # dff tiles
            hps = psB.tile([128, 512], F32, tag="psB")
            for k in range(8):            # contraction over hid chunks
                nc.tensor.matmul(
                    hps[:, 0:256],
                    w1_sb[:, k * DFF + m * 128: k * DFF + (m + 1) * 128],
                    liT[:, k * 256:(k + 1) * 256],
                    start=(k == 0), stop=(k == 7))
            h_m = hp.tile([128, 256], BF16, tag="h")
            nc.scalar.activation(h_m[:, :], hps[:, 0:256],
                                 mybir.ActivationFunctionType.Gelu_apprx_tanh,
                                 bias=b1_sb[:, m:m + 1])
            for ti in range(2):
                for hf in range(2):
                    nc.tensor.matmul(
                        fps[2 * ti + hf][:, :],
                        h_m[:, ti * 128:(ti + 1) * 128],
                        w2_sb[:, m * HID + hf * 512: m * HID + (hf + 1) * 512],
                        start=(m == 0), stop=False)
        for ti in range(2):
            for hf in range(2):
                nc.tensor.matmul(fps[2 * ti + hf][:, :], ones_sb[:, :],
                                 b2_sb[:, hf * 512:(hf + 1) * 512],
                                 start=False, stop=True)
        for ti in range(2):
            r0 = (2 * g + ti) * 128
            # per-token int8 quant: rmax -> inv -> out8 = fps * inv * QMAX
            q_sb = qp.tile([128, 8], F32, tag="q")
            nc.vector.tensor_reduce(q_sb[:, 0:1], fps[2 * ti][:, :],
                                    mybir.AxisListType.X,
                                    mybir.AluOpType.max,
                                    apply_absolute_value=True)
            nc.vector.tensor_reduce(q_sb[:, 1:2], fps[2 * ti + 1][:, :],
                                    mybir.AxisListType.X,
                                    mybir.AluOpType.max,
                                    apply_absolute_value=True)
            nc.vector.tensor_max(q_sb[:, 2:3], q_sb[:, 0:1], q_sb[:, 1:2])
            nc.vector.tensor_scalar_max(q_sb[:, 3:4], q_sb[:, 2:3], 1e-20)
            nc.vector.reciprocal(q_sb[:, 4:5], q_sb[:, 3:4])
            o8 = outp.tile([128, HID], I8, tag="out8")
            nc.vector.tensor_scalar(o8[:, 0:512], fps[2 * ti][:, :],
                                    q_sb[:, 4:5], QMAX, MULT, MULT)
            nc.vector.tensor_scalar(o8[:, 512:1024], fps[2 * ti + 1][:, :],
                                    q_sb[:, 4:5], QMAX, MULT, MULT)
            nc.scalar.activation(q_sb[:, 5:6], q_sb[:, 3:4],
                                 mybir.ActivationFunctionType.Copy,
                                 scale=1.0 / QMAX)
            nc.sync.dma_start(out8_d[r0:r0 + 128, :], o8[:, :])
            nc.sync.dma_start(outs_d[r0:r0 + 128, :], q_sb[:, 5:6])

    for p in reversed(pools):
        p.release()


def _build_runner(nc):
    install_neuronx_cc_hook()
    devices = jax.devices()[:N_CORES]
    assert len(devices) == N_CORES
    mesh = Mesh(np.asarray(devices), ("core",))
    sh = NamedSharding(mesh, PartitionSpec("core"))

    partition_name = (nc.partition_id_tensor.name
                      if nc.partition_id_tensor is not None else None)
    in_names, out_names, out_avals = [], [], []
    for alloc in nc.m.functions[0].allocations:
        if not isinstance(alloc, mybir.MemoryLocationSet):
            continue
        name = alloc.memorylocations[0].name
        if alloc.kind == "ExternalInput":
            if name != partition_name:
                in_names.append(name)
        elif alloc.kind == "ExternalOutput":
            out_names.append(name)
            out_avals.append(jax.core.ShapedArray(
                tuple(alloc.tensor_shape), mybir.dt.np(alloc.dtype)))
    assert nc.dbg_addr is None
    assert in_names == ["li8", "lis"] and out_names == ["out8", "outs"], \
        (in_names, out_names)
    n_params = len(in_names)
    n_outs = len(out_names)
    all_in_names = tuple(in_names + out_names +
                         ([partition_name] if partition_name else []))

    def _body(*args):
        operands = list(args)
        if partition_name is not None:
            operands.append(partition_id_tensor())
        outs = _bass_exec_p.bind(
            *operands,
            out_avals=tuple(out_avals),
            in_names=all_in_names,
            out_names=tuple(out_names),
            lowering_input_output_aliases=(),
            sim_require_finite=True,
            sim_require_nnan=True,
            nc=nc,
        )
        return tuple(outs)

    jitted = jax.jit(
        shard_map(_body, mesh=mesh,
                  in_specs=(PartitionSpec("core"),) * (n_params + n_outs),
                  out_specs=(PartitionSpec("core"),) * n_outs,
                  check_rep=False),
        donate_argnums=tuple(range(n_params, n_params + n_outs)),
        keep_unused=True)

    out_global = [(tuple([N_CORES * a.shape[0]] + list(a.shape[1:])), a.dtype)
                  for a in out_avals]

    def _zeros():
        return tuple(jnp.zeros(s, d) for s, d in out_global)

    zeros_jit = jax.jit(_zeros, out_shardings=(sh,) * n_outs)
    return dict(sh=sh, jitted=jitted, zeros_jit=zeros_jit,
                devices=list(devices))


def _fingerprint(*arrays):
    h = hashlib.sha1()
    for a in arrays:
        a = np.asarray(a)
        h.update(repr((a.shape, str(a.dtype))).encode())
        flat = a.reshape(-1)
        step = max(1, flat.size // 65536)
        h.update(np.ascontiguousarray(flat[::step]).tobytes())
    return h.hexdigest()


def _get_state(ffn_w1, ffn_b1, ffn_w2, ffn_b2):
    key = _fingerprint(ffn_w1, ffn_b1, ffn_w2, ffn_b2)
    st = _STATE.get(key)
    if st is None:
        w1t = np.ascontiguousarray(
            np.asarray(ffn_w1, np.float32).T).astype(BF)          # [1024, 4096]
        w2t = np.ascontiguousarray(
            np.asarray(ffn_w2, np.float32).T).astype(BF)          # [4096, 1024]
        b1r = np.ascontiguousarray(
            np.asarray(ffn_b1, np.float32).reshape(32, 128).T)    # [128, 32]
        b2r = np.asarray(ffn_b2, np.float32).reshape(1, HID).astype(BF)
        nc = _build_module(w1t, w2t, b1r, b2r)
        st = _build_runner(nc)
        st["nc"] = nc
        _STATE[key] = st
    return st


def _sigmoid(z):
    return 1.0 / (1.0 + np.exp(-z))


def _buffers():
    b = _STATE.get("bufs")
    if b is None:
        b = dict(
            li=np.empty((NTOK, HID), np.float32),
            tmp=np.empty((NTOK, HID), np.float32),
            li8=np.empty((NTOK, HID), np.int8),
            tmp2=np.empty((NTOK, HID), np.float32),
            lis=np.empty((NTOK, 1), np.float32),
            proj=np.empty((NTOK, 32), np.float32),
        )
        _STATE["bufs"] = b
    return b


def kernel(x_streams, alpha_pre, alpha_post, alpha_res,
           W_pre_w, W_pre_b, W_post_w, W_post_b, W_res_w, W_res_b,
           ffn_w1, ffn_b1, ffn_w2, ffn_b2, perm_mat):
    t0 = time.perf_counter()
    st = _get_state(ffn_w1, ffn_b1, ffn_w2, ffn_b2)
    bufs = _buffers()

    x = np.ascontiguousarray(np.asarray(x_streams, np.float32)) \
        .reshape(NTOK, NCH)
    x4 = x.reshape(NTOK, 4, HID)

    wcatT = np.ascontiguousarray(np.concatenate(
        [np.asarray(W_pre_w, np.float32),
         np.asarray(W_post_w, np.float32),
         np.asarray(W_res_w, np.float32)], axis=0).T)    # [NCH, 32]
    bcat = np.concatenate([np.asarray(W_pre_b, np.float32),
                           np.asarray(W_post_b, np.float32),
                           np.asarray(W_res_b, np.float32)])
    a_pre = float(np.asarray(alpha_pre).reshape(-1)[0])
    a_post = float(np.asarray(alpha_post).reshape(-1)[0])
    a_res = float(np.asarray(alpha_res).reshape(-1)[0])

    zeros = st["zeros_jit"]()                            # async device fill
    li = bufs["li"]
    tmp = bufs["tmp"]
    li8 = bufs["li8"]
    lis = bufs["lis"]
    proj = bufs["proj"]
    # rms-norm scale; projections computed as (x @ W.T) * s + b
    ssq = np.einsum('ij,ij->i', x, x)
    s = 1.0 / np.sqrt(ssq * (1.0 / NCH) + EPS)
    np.matmul(x, wcatT, out=proj)
    proj *= s[:, None]
    proj += bcat
    h_pre = _sigmoid(a_pre * proj[:, 0:4])
    np.multiply(h_pre[:, 0:1], x4[:, 0, :], out=li)      # layer_input
    for j in range(1, 4):
        np.multiply(h_pre[:, j:j + 1], x4[:, j, :], out=tmp)
        li += tmp
    # int8 quantize with per-token scale
    r = np.maximum(li.max(axis=1), -li.min(axis=1))
    np.maximum(r, 1e-20, out=r)
    np.multiply(li, (QMAX / r)[:, None], out=tmp)
    np.rint(tmp, out=tmp)
    np.copyto(li8, tmp, casting='unsafe')
    lis[:, 0] = r
    lis *= 1.0 / QMAX
    t1 = time.perf_counter()

    # dispatch the device FFN (async), overlap remaining host math
    li8_dev, lis_dev = jax.device_put((li8, lis), st["sh"])
    out_dev = st["jitted"](li8_dev, lis_dev, *zeros)
    t2 = time.perf_counter()

    h_post = 2.0 * _sigmoid(a_post * proj[:, 4:8])
    z = a_res * proj[:, 8:32]
    z -= z.max(axis=1, keepdims=True)
    np.exp(z, out=z)
    z /= z.sum(axis=1, keepdims=True)
    Hm = (z @ np.asarray(perm_mat, np.float32)).reshape(NTOK, 4, 4)
    y = np.matmul(Hm, x4)                                # mixed, fp32 x
    t3 = time.perf_counter()

    out8, outs = jax.device_get(out_dev)                 # blocks on fetch
    t4 = time.perf_counter()
    d = tmp                                              # reuse scratch
    np.multiply(out8, outs, out=d)                       # dequant ffn_out
    np.subtract(d, li, out=d)                            # delta
    tmp2 = bufs["tmp2"]
    for i in range(4):
        np.multiply(h_post[:, i:i + 1], d, out=tmp2)
        y[:, i, :] += tmp2
    t5 = time.perf_counter()
    if _DBG:
        print(f"[kt] prologue {t1-t0:.3f}s dispatch {t2-t1:.3f}s "
              f"overlap {t3-t2:.3f}s fetch {t4-t3:.3f}s epilogue {t5-t4:.3f}s"
              f" total {t5-t0:.3f}s", flush=True)
    return y.reshape(4, 2048, 4, 1024)
